# revision 17
# baseline (speedup 1.0000x reference)
"""Trainium2 Bass kernel for MixtureOfAttentionLayer (B=4, S=1024, H=1024,
E=4 attention experts [std-8h, std-12h, linear-8h, local-8h], top-2 gating).

Sharding: 8 cores; core c -> batch b=c//2, head-half p=c%2. Each core computes
its half of every expert's heads for its batch; the host sums core pairs.
Gating is computed on the host and exploited for *query sparsification*:
each expert only receives ~half the tokens (top-2 of 4), so for experts
0/1/2 the host gathers the selected tokens and the device runs the whole
q-side (Q projection, scores, exp, PV, denominator, out-projection) on a
packed [S_pad] token axis (S_pad = max selected count rounded up to 128,
computed at runtime). K/V stay dense (attention needs all keys). Expert 3
(local/banded) stays fully dense since gathering breaks the band structure.
The host scatter-adds the per-expert partial outputs (shipped as bf16).

Device dataflow (all matmuls bf16 inputs, f32 PSUM):
  xT [H,S] -> QT/KT [pd,S*] and V [S,pd] projections (biases via K=1 matmuls;
  bk dropped for softmax experts - a row-constant score shift is
  softmax-invariant; bv folded into a host-computed effective output bias).
  Scores are computed transposed ST=[k,q] (no max-subtraction: score scale is
  tiny so exp is safe), exp on ScalarE, PV gives OT=[hd,q] directly. For
  experts 0/3 (hd=128) the softmax denominator is produced already broadcast
  across partitions by a matmul with an all-ones [128,128] stationary
  operand. For expert 1 (hd=85) the packed V carries a ones-column at
  position 0 of each 86-wide head slice, so the PV matmul emits the
  denominator as PSUM row 0 for free; a K=1 matmul broadcasts it across
  partitions, and Wo carries matching zero rows so the junk row dies in the
  out-projection. Local attention is banded (<=4 k-tiles per 256-query
  chunk) with precomputed 0/1 mask tiles. All DRAM parameters are
  pre-permuted on the host to [128, hk, ...] so every DMA is contiguous
  per partition.
"""
import os
import sys
import math
import functools

import numpy as np

for _p in ("/root/.axon_site/_ro/trn_rl_repo", "/opt/trn_rl_repo"):
    if os.path.isdir(_p) and _p not in sys.path:
        sys.path.insert(0, _p)

import types

if "antenv.axon_hooks" not in sys.modules:
    # The image's read-only antenv package lacks axon_hooks; seed it so
    # trn_boot can register the NTFF profile hook (used when trace=True).
    _m = types.ModuleType("antenv.axon_hooks")
    _m._hook = None

    def _set_hook(h, _m=_m):
        _m._hook = h

    def _get_hook(_m=_m):
        return _m._hook

    _m.set_axon_ntff_profile_hook = _set_hook
    _m.get_axon_ntff_profile_hook = _get_hook
    sys.modules["antenv.axon_hooks"] = _m

import ml_dtypes

BF16 = ml_dtypes.bfloat16

P = 128
S = 1024
H = 1024
E = 4
HK = H // P  # 8 H-tiles
NH = [8, 12, 8, 8]
HD = [128, 85, 128, 128]
ATYPE = ["std", "std", "lin", "loc"]
NHC = [4, 6, 4, 4]          # heads per core
PDC = [512, 768, 512, 512]  # padded per-core concat head dim (QT/KT/Wo layout)
PDV = [512, 516, 512, 512]  # packed per-core V width (e1: 6 x 86, ones col 0)
WHALF = 32
N_CORES = 8
SPARSE = (0, 1, 2)          # query-sparsified experts


# ---------------------------------------------------------------- host prep

def _host_gates(x_b, Wg):
    """x_b [S,H] f32, Wg [H,E] -> gatesT [E,S] f32 (0 for unselected)."""
    logits = x_b @ Wg  # [S, E]
    srt = np.sort(logits, axis=1)
    m1 = srt[:, -1]
    m2 = srt[:, -2]
    den = 1.0 + np.exp(m2 - m1)
    w = np.exp(logits - m1[:, None]) / den[:, None]
    w = np.where(logits >= m2[:, None], w, 0.0)
    return np.ascontiguousarray(w.T.astype(np.float32))  # [E, S]


def _perm_h(W):
    """[n*128, width] row-tiled -> [128, n, width] contiguous per partition."""
    n = W.shape[0] // P
    return np.ascontiguousarray(
        W.reshape(n, P, W.shape[1]).transpose(1, 0, 2)).astype(BF16)


def _pad_cols(W, hd, heads):
    out = np.zeros((W.shape[0], len(heads) * P), np.float32)
    for i, h in enumerate(heads):
        out[:, i * P : i * P + hd] = W[:, h * hd : (h + 1) * hd]
    return out


def _pad_rows(W, hd, heads):
    out = np.zeros((len(heads) * P, W.shape[1]), np.float32)
    for i, h in enumerate(heads):
        out[i * P : i * P + hd] = W[h * hd : (h + 1) * hd]
    return out


def _pad_rows_sh1(W, hd, heads):
    """Wo rows shifted by +1 within each 128-tile (row 0 = denominator junk)."""
    out = np.zeros((len(heads) * P, W.shape[1]), np.float32)
    for i, h in enumerate(heads):
        out[i * P + 1 : i * P + 1 + hd] = W[h * hd : (h + 1) * hd]
    return out


def _pad_vec(v, hd, heads):
    out = np.zeros((len(heads) * P,), np.float32)
    for i, h in enumerate(heads):
        out[i * P : i * P + hd] = v[h * hd : (h + 1) * hd]
    return out


def _band_masks():
    masks = np.zeros((P, 3, P), np.float32)
    for mi, delta in enumerate((-128, 0, 128)):
        pp = np.arange(P)[:, None]
        ff = np.arange(P)[None, :]
        masks[:, mi, :] = (np.abs(delta + pp - ff) <= WHALF).astype(np.float32)
    return masks.astype(BF16)


def _prep_core(inputs, b, p, masks, gatesT, sel_idx, s_pad):
    d = {}
    x_b = inputs["x"][b]  # [S, H] f32
    d["x_t"] = _perm_h(np.ascontiguousarray(x_b.T))
    for e in SPARSE:
        xs = np.zeros((s_pad, H), np.float32)
        sel = sel_idx[e]
        xs[: len(sel)] = x_b[sel]
        d[f"xq{e}"] = _perm_h(np.ascontiguousarray(xs.T))
        gsel = np.zeros((s_pad,), np.float32)
        gsel[: len(sel)] = gatesT[e][sel]
        d[f"gbq{e}"] = np.ascontiguousarray(
            np.broadcast_to(gsel[None, :], (P, s_pad))).astype(np.float32)
    d["gb3"] = np.ascontiguousarray(
        np.broadcast_to(gatesT[3][None, :], (P, S))).astype(np.float32)
    d["masks"] = masks
    for e in range(E):
        hd, nhc = HD[e], NHC[e]
        heads = list(range(p * nhc, (p + 1) * nhc))
        scale = 1.0 / math.sqrt(hd) if ATYPE[e] in ("std", "loc") else 1.0
        if e == 1:
            # packed projections (4 chains instead of 6); repacked to the
            # per-head-padded layout on device via partition-remap DMAs
            wqp = np.zeros((H, 512), np.float32)
            wkp = np.zeros((H, 512), np.float32)
            bqp = np.zeros((512,), np.float32)
            for i, h in enumerate(heads):
                wqp[:, i * hd : (i + 1) * hd] = inputs["e1_Wq"][:, h * hd : (h + 1) * hd]
                wkp[:, i * hd : (i + 1) * hd] = inputs["e1_Wk"][:, h * hd : (h + 1) * hd]
                bqp[i * hd : (i + 1) * hd] = inputs["e1_bq"][h * hd : (h + 1) * hd]
            d["wq1"] = _perm_h(wqp * scale)
            d["wk1"] = _perm_h(wkp)
            d["bqc1"] = np.ascontiguousarray(
                (bqp * scale).reshape(4, P).T).astype(np.float32)
        else:
            d[f"wq{e}"] = _perm_h(_pad_cols(inputs[f"e{e}_Wq"], hd, heads) * scale)
            bqp = _pad_vec(inputs[f"e{e}_bq"], hd, heads) * scale
            d[f"bqc{e}"] = np.ascontiguousarray(
                bqp.reshape(-1, P).T).astype(np.float32)  # [P, pdc//P]
            d[f"wk{e}"] = _perm_h(_pad_cols(inputs[f"e{e}_Wk"], hd, heads))
        if e == 1:
            wv = np.zeros((H, PDV[1]), np.float32)
            for i, h in enumerate(heads):
                wv[:, i * 86 + 1 : i * 86 + 86] = \
                    inputs["e1_Wv"][:, h * hd : (h + 1) * hd]
            d["wv1"] = _perm_h(wv)
            d["wo1"] = _perm_h(_pad_rows_sh1(inputs["e1_Wo"], hd, heads))
        else:
            d[f"wv{e}"] = _perm_h(np.ascontiguousarray(
                inputs[f"e{e}_Wv"][:, heads[0] * hd : (heads[-1] + 1) * hd]))
            d[f"wo{e}"] = _perm_h(_pad_rows(inputs[f"e{e}_Wo"], hd, heads))
        if e == 2:
            d["bk2"] = np.ascontiguousarray(
                _pad_vec(inputs["e2_bk"], hd, heads)[None, :]).astype(BF16)
    return d


# ---------------------------------------------------------------- device IR

@functools.lru_cache(maxsize=2)
def _build_nc(s_pad):
    import concourse.mybir as mybir
    import concourse.tile as tile
    from concourse import bacc

    f32 = mybir.dt.float32
    bf16 = mybir.dt.bfloat16
    Exp = mybir.ActivationFunctionType.Exp
    Copy = mybir.ActivationFunctionType.Copy
    Ident = mybir.ActivationFunctionType.Identity

    nc = bacc.Bacc(None, target_bir_lowering=False)

    x_t = nc.declare_dram_parameter("x_t", [P, HK, S], bf16, isOutput=False)
    xq_d = {e: nc.declare_dram_parameter(f"xq{e}", [P, HK, s_pad], bf16,
                                         isOutput=False) for e in SPARSE}
    gbq_d = {e: nc.declare_dram_parameter(f"gbq{e}", [P, s_pad], f32,
                                          isOutput=False) for e in SPARSE}
    gb3_d = nc.declare_dram_parameter("gb3", [P, S], f32, isOutput=False)
    masks_d = nc.declare_dram_parameter("masks", [P, 3, P], bf16, isOutput=False)
    PDP = [512, 512, 512, 512]  # packed projection widths (e1 packs 6x85->512)
    wq_d, wk_d, wv_d, wo_d, bq_d = [], [], [], [], []
    for e in range(E):
        wq_d.append(nc.declare_dram_parameter(f"wq{e}", [P, HK, PDP[e]], bf16, isOutput=False))
        wk_d.append(nc.declare_dram_parameter(f"wk{e}", [P, HK, PDP[e]], bf16, isOutput=False))
        wv_d.append(nc.declare_dram_parameter(f"wv{e}", [P, HK, PDV[e]], bf16, isOutput=False))
        wo_d.append(nc.declare_dram_parameter(f"wo{e}", [P, PDC[e] // P, H], bf16, isOutput=False))
        bq_d.append(nc.declare_dram_parameter(f"bqc{e}", [P, PDP[e] // P], f32, isOutput=False))
    bk2_d = nc.declare_dram_parameter("bk2", [1, PDC[2]], bf16, isOutput=False)
    out_d = [nc.declare_dram_parameter(f"out{e}", [s_pad if e in SPARSE else S, H],
                                       bf16, isOutput=True) for e in range(E)]
    out_r = [o.ap().rearrange("(o p) h -> p o h", p=P) for o in out_d]

    SCH = [(0, 512), (512, 512)]  # dense S chunks
    SCHQ = [(0, min(512, s_pad))]
    if s_pad > 512:
        SCHQ.append((512, s_pad - 512))
    NQT = s_pad // P

    def pd_chunks(pdc):
        out, off = [], 0
        while off < pdc:
            w = min(512, pdc - off)
            out.append((off, w))
            off += w
        return out

    with tile.TileContext(nc) as tc:
        with (
            tc.tile_pool(name="singles", bufs=1) as singles,
            tc.tile_pool(name="wpool", bufs=1) as wpool,
            tc.tile_pool(name="xqpool", bufs=1) as xqpool,
            tc.tile_pool(name="qpool", bufs=2) as qpool,
            tc.tile_pool(name="opool", bufs=4) as opool,
            tc.tile_pool(name="apool", bufs=1) as apool,
            tc.tile_pool(name="otpool", bufs=1) as otpool,
            tc.tile_pool(name="epool", bufs=2) as epool,
            tc.tile_pool(name="tpool", bufs=2) as tpool,
            tc.tile_pool(name="psA", bufs=4, space="PSUM") as psA,
            tc.tile_pool(name="psB", bufs=2, space="PSUM") as psB,
            tc.tile_pool(name="psC", bufs=2, space="PSUM") as psC,
        ):
            # ---- tiny constants first (cheap DMAs, then PE-critical loads
            # are issued at the top of each expert section)
            bq_sb = []
            for e in range(E):
                t = singles.tile([P, PDP[e] // P], f32, name=f"bqc_sb{e}")
                nc.sync.dma_start(t[:], bq_d[e].ap())
                bq_sb.append(t)
            bk2_sb = singles.tile([1, PDC[2]], bf16)
            nc.sync.dma_start(bk2_sb[:], bk2_d.ap())

            ones_row = singles.tile([1, S], bf16)
            nc.vector.memset(ones_row[:], 1.0)
            ones_col = singles.tile([P, 1], bf16)
            nc.vector.memset(ones_col[:], 1.0)
            ones_mat = singles.tile([P, P], bf16)
            nc.vector.memset(ones_mat[:], 1.0)

            OT = [otpool.tile([P, PDC[e] // P, s_pad if e in SPARSE else S],
                              bf16, name=f"ot{e}") for e in range(E)]

            xT = singles.tile([P, HK, S], bf16)

            def load_hk(dram, t, width):
                """[P, HK, width] dram -> sbuf, split per hk-pair for
                early partial-dependency starts."""
                r = dram.ap()
                for q in range(4):
                    nc.sync.dma_start(t[:, 2 * q : 2 * q + 2],
                                      r[:, 2 * q : 2 * q + 2])
                return t

            def load_w(e, which, dram, pdc):
                t = wpool.tile([P, HK, pdc], bf16, tag=which, name=f"{which}{e}")
                return load_hk(dram, t, pdc)

            def proj_T(w_sb, pdc, xsrc, chunks):
                """QT/KT-style projection psums: [P(d-cols), chunk]."""
                for ht in range(pdc // P):
                    for (c0, cw) in chunks:
                        ps = psA.tile([P, 512], f32, tag="mm", name="proj_ps")
                        for hk in range(HK):
                            nc.tensor.matmul(
                                ps[:, :cw],
                                w_sb[:, hk, ht * P : (ht + 1) * P],
                                xsrc[:, hk, c0 : c0 + cw],
                                start=(hk == 0),
                                stop=(hk == HK - 1),
                            )
                        yield ps, ht, c0, cw

            def proj_nat(w_sb, pdc, bias_sb=None):
                """V-style natural projection psums: [P(s), chunk] = xT.T @ W."""
                for st in range(HK):
                    for (c0, cw) in pd_chunks(pdc):
                        ps = psA.tile([P, 512], f32, tag="mm", name="projn_ps")
                        for hk in range(HK):
                            nc.tensor.matmul(
                                ps[:, :cw],
                                xT[:, hk, st * P : (st + 1) * P],
                                w_sb[:, hk, c0 : c0 + cw],
                                start=(hk == 0),
                                stop=(hk == HK - 1 and bias_sb is None),
                            )
                        if bias_sb is not None:
                            nc.tensor.matmul(
                                ps[:, :cw],
                                ones_row[:, :P],
                                bias_sb[:, c0 : c0 + cw],
                                start=False, stop=True,
                            )
                        yield ps, st, c0, cw

            def elu_p1(ps, dst_ap, cw, bias=None):
                """dst = elu(ps + bias)+1 = exp(min(.,0)) + max(.,0), bf16."""
                tmin = tpool.tile([P, 512], f32, tag="tmin", name="tmin")
                texp = tpool.tile([P, 512], f32, tag="texp", name="texp")
                tmax = tpool.tile([P, 512], f32, tag="tmin", name="tmax")
                if bias is None:
                    nc.vector.tensor_scalar_min(tmin[:, :cw], ps[:, :cw], 0.0)
                    nc.vector.tensor_scalar_max(tmax[:, :cw], ps[:, :cw], 0.0)
                else:
                    nc.vector.tensor_scalar(
                        tmin[:, :cw], ps[:, :cw], bias, 0.0,
                        mybir.AluOpType.add, mybir.AluOpType.min)
                    nc.vector.tensor_scalar(
                        tmax[:, :cw], ps[:, :cw], bias, 0.0,
                        mybir.AluOpType.add, mybir.AluOpType.max)
                nc.scalar.activation(texp[:, :cw], tmin[:, :cw], Exp)
                nc.vector.tensor_add(dst_ap, texp[:, :cw], tmax[:, :cw])

            def gated_norm(den_ps, gb_sb, c0, cw, num_ps, out_ap, np_=P):
                """out = num * (1/den) * gate_w ; den_ps replicated [np_, cw]."""
                rcp = tpool.tile([P, 512], f32, tag="rcp", name="rcp")
                nc.vector.reciprocal_approx_fast(out=rcp[:np_, :cw], in_=den_ps[:np_, :cw])
                tnum = tpool.tile([P, 512], f32, tag="tnum", name="tnum")
                nc.vector.tensor_mul(tnum[:np_, :cw], num_ps[:np_, :cw], rcp[:np_, :cw])
                nc.vector.tensor_mul(out_ap, tnum[:np_, :cw], gb_sb[:np_, c0 : c0 + cw])

            def outproj(e, n_st):
                wo = wpool.tile([P, PDC[e] // P, H], bf16, tag="wo", name=f"wo{e}")
                for pt in range(PDC[e] // P):
                    nc.sync.dma_start(wo[:, pt : pt + 1], wo_d[e].ap()[:, pt : pt + 1])
                for st in range(n_st):
                    for (c0, cw) in SCH:
                        ps = psA.tile([P, 512], f32, tag="mm", name="out_ps")
                        npt = PDC[e] // P
                        for pt in range(npt):
                            nc.tensor.matmul(
                                ps[:, :cw],
                                OT[e][:, pt, st * P : (st + 1) * P],
                                wo[:, pt, c0 : c0 + cw],
                                start=(pt == 0), stop=(pt == npt - 1),
                            )
                        o_sb = opool.tile([P, 512], bf16, tag="osb", name="o_sb")
                        nc.vector.tensor_copy(o_sb[:, :cw], ps[:, :cw])
                        nc.sync.dma_start(out_r[e][:, st, c0 : c0 + cw], o_sb[:, :cw])

            first = True
            # ================= per-expert compute =================
            # order ends on e2 (smallest out-projection -> short DMA tail)
            for e in (0, 3, 1, 2):
                pdc = PDC[e]
                pdv = PDV[e]
                nhc = NHC[e]
                sparse = e in SPARSE
                chq = SCHQ if sparse else SCH
                sq = s_pad if sparse else S

                wq = load_w(e, "wq", wq_d[e], PDP[e])
                if sparse:
                    xq = xqpool.tile([P, HK, s_pad], bf16, tag="xq", name=f"xq{e}")
                    load_hk(xq_d[e], xq, s_pad)
                    xsrc = xq
                else:
                    xsrc = xT
                if first:
                    # xT after the e0 Q-path loads: needed from K-proj on
                    load_hk(x_t, xT, S)
                    first = False
                wk = load_w(e, "wk", wk_d[e], PDP[e])
                wv = load_w(e, "wv", wv_d[e], pdv)

                # per-token gate weight, pre-broadcast across partitions
                gb_sb = apool.tile([P, sq], f32, tag="gb", name="gb")
                nc.sync.dma_start(gb_sb[:], (gbq_d[e] if sparse else gb3_d).ap())

                if e == 1:
                    # packed projections, then partition-remap (DMA) into the
                    # per-head 128-padded layout; pad rows zeroed by memset
                    QT = qpool.tile([P, pdc // P, sq], bf16, tag="qt", name="QT")
                    nc.vector.memset(QT[:], 0.0)
                    KT = apool.tile([P, pdc // P, S], bf16, tag="kt", name="KT")
                    nc.vector.memset(KT[:], 0.0)
                    QTP = apool.tile([P, 4, sq], bf16, tag="qtp", name="QTP")
                    for ps, ht, c0, cw in proj_T(wq, 512, xsrc, chq):
                        nc.scalar.activation(
                            QTP[:, ht, c0 : c0 + cw], ps[:, :cw], Ident,
                            bias=bq_sb[e][:, ht : ht + 1])
                    KTP = apool.tile([P, 4, S], bf16, tag="ktp", name="KTP")
                    for ps, ht, c0, cw in proj_T(wk, 512, xT, SCH):
                        nc.scalar.activation(KTP[:, ht, c0 : c0 + cw], ps[:, :cw], Copy)
                    for h in range(nhc):
                        r0 = 85 * h
                        t0, o0 = r0 // P, r0 % P
                        l1 = min(P - o0, 85)
                        nc.sync.dma_start(QT[0:l1, h], QTP[o0 : o0 + l1, t0])
                        nc.sync.dma_start(KT[0:l1, h], KTP[o0 : o0 + l1, t0])
                        if l1 < 85:
                            nc.sync.dma_start(QT[l1:85, h], QTP[0 : 85 - l1, t0 + 1])
                            nc.sync.dma_start(KT[l1:85, h], KTP[0 : 85 - l1, t0 + 1])
                elif ATYPE[e] in ("std", "loc"):
                    QT = qpool.tile([P, pdc // P, sq], bf16, tag="qt", name="QT")
                    for ps, ht, c0, cw in proj_T(wq, pdc, xsrc, chq):
                        nc.scalar.activation(
                            QT[:, ht, c0 : c0 + cw], ps[:, :cw], Ident,
                            bias=bq_sb[e][:, ht : ht + 1])
                    KT = apool.tile([P, pdc // P, S], bf16, tag="kt", name="KT")
                    for ps, ht, c0, cw in proj_T(wk, pdc, xT, SCH):
                        nc.scalar.activation(KT[:, ht, c0 : c0 + cw], ps[:, :cw], Copy)
                else:  # linear: q' = elu(QT+bq)+1 ; k' natural = elu(K+bk)+1
                    QT = qpool.tile([P, pdc // P, sq], bf16, tag="qt", name="QTl")
                    for ps, ht, c0, cw in proj_T(wq, pdc, xsrc, chq):
                        elu_p1(ps, QT[:, ht, c0 : c0 + cw], cw,
                               bias=bq_sb[e][:, ht : ht + 1])
                    KT = apool.tile([P, HK, pdc], bf16, tag="kt", name="Kn")
                    for ps, st, c0, cw in proj_nat(wk, pdc, bias_sb=bk2_sb):
                        elu_p1(ps, KT[:, st, c0 : c0 + cw], cw)
                V = apool.tile([P, HK, pdv], bf16, tag="v", name="V")
                for ps, st, c0, cw in proj_nat(wv, pdv):
                    nc.scalar.activation(V[:, st, c0 : c0 + cw], ps[:, :cw], Copy)
                if e == 1:
                    # ones column at position 0 of each 86-wide head slice
                    # (projection wrote zeros there: wv1 host cols are zero)
                    for h in range(nhc):
                        nc.vector.memset(V[:, :, h * 86 : h * 86 + 1], 1.0)
                    # OT1 rows 86:128 per head tile are never written; wo1 has
                    # zero rows there but stale NaNs would still poison 0*NaN
                    nc.vector.memset(OT[1][:], 0.0)

                if e == 1:  # std attention, denominator via V ones-column
                    # den_bc + gated_norm are deferred one iteration so the
                    # ACT den-row copy never sits on the PE critical path
                    def flush1(pend):
                        o_ps, den_sb, c0, cw, h = pend
                        den = psC.tile([P, 512], f32, tag="den", name="den_bc")
                        nc.tensor.matmul(
                            den[:, :cw], ones_row[:, :P], den_sb[:, :cw],
                            start=True, stop=True,
                        )
                        gated_norm(den, gb_sb, c0, cw, o_ps,
                                   OT[1][:86, h, c0 : c0 + cw], np_=86)

                    pend = None
                    for h in range(nhc):
                        for (c0, cw) in chq:
                            est = epool.tile([P, HK, 512], bf16, tag="est", name="est")
                            for kt in range(HK):
                                st_ps = psA.tile([P, 512], f32, tag="mm", name="st_ps")
                                nc.tensor.matmul(
                                    st_ps[:, :cw],
                                    KT[:, h, kt * P : (kt + 1) * P],
                                    QT[:, h, c0 : c0 + cw],
                                    start=True, stop=True,
                                )
                                nc.scalar.activation(
                                    est[:, kt, :cw], st_ps[:, :cw], Exp)
                            o_ps = psB.tile([P, 512], f32, tag="ot", name="o_ps")
                            for kt in range(HK):
                                nc.tensor.matmul(
                                    o_ps[:86, :cw],
                                    V[:, kt, h * 86 : (h + 1) * 86],
                                    est[:, kt, :cw],
                                    start=(kt == 0), stop=(kt == HK - 1),
                                )
                            den_sb = tpool.tile([1, 512], bf16, tag="densb", name="den_sb")
                            nc.scalar.activation(den_sb[:, :cw], o_ps[0:1, :cw], Copy)
                            if pend is not None:
                                flush1(pend)
                            pend = (o_ps, den_sb, c0, cw, h)
                    flush1(pend)

                elif ATYPE[e] == "std":
                    for h in range(nhc):
                        for (c0, cw) in chq:
                            est = epool.tile([P, HK, 512], bf16, tag="est", name="est")
                            for kt in range(HK):
                                st_ps = psA.tile([P, 512], f32, tag="mm", name="st_ps")
                                nc.tensor.matmul(
                                    st_ps[:, :cw],
                                    KT[:, h, kt * P : (kt + 1) * P],
                                    QT[:, h, c0 : c0 + cw],
                                    start=True, stop=True,
                                )
                                nc.scalar.activation(
                                    est[:, kt, :cw], st_ps[:, :cw], Exp)
                            o_ps = psB.tile([P, 512], f32, tag="ot", name="o_ps")
                            den = psC.tile([P, 512], f32, tag="den", name="den")
                            for kt in range(HK):
                                nc.tensor.matmul(
                                    o_ps[:, :cw],
                                    V[:, kt, h * P : (h + 1) * P],
                                    est[:, kt, :cw],
                                    start=(kt == 0), stop=(kt == HK - 1),
                                )
                            for kt in range(HK):
                                nc.tensor.matmul(
                                    den[:, :cw],
                                    ones_mat[:],
                                    est[:, kt, :cw],
                                    start=(kt == 0), stop=(kt == HK - 1),
                                )
                            gated_norm(den, gb_sb, c0, cw, o_ps,
                                       OT[e][:, h, c0 : c0 + cw])

                elif ATYPE[e] == "loc":
                    masks_sb = singles.tile([P, 3, P], bf16)
                    nc.sync.dma_start(masks_sb[:], masks_d.ap())
                    for h in range(nhc):
                        for t in range(HK):
                            kts = [kt for kt in (t - 1, t, t + 1)
                                   if 0 <= kt < HK]
                            est = epool.tile([P, 3, P], bf16, tag="estl", name="estl")
                            for kt in kts:
                                mi = kt - t + 1
                                st_ps = psA.tile([P, 512], f32, tag="mm", name="stl_ps")
                                nc.tensor.matmul(
                                    st_ps[:, :P],
                                    KT[:, h, kt * P : (kt + 1) * P],
                                    QT[:, h, t * P : (t + 1) * P],
                                    start=True, stop=True,
                                )
                                nc.scalar.activation(
                                    est[:, mi, :], st_ps[:, :P], Exp)
                                nc.vector.tensor_mul(
                                    est[:, mi, :], est[:, mi, :], masks_sb[:, mi, :])
                            o_ps = psB.tile([P, 512], f32, tag="ot", name="ol_ps")
                            den = psC.tile([P, 512], f32, tag="den", name="denl")
                            for i, kt in enumerate(kts):
                                mi = kt - t + 1
                                nc.tensor.matmul(
                                    o_ps[:, :P],
                                    V[:, kt, h * P : (h + 1) * P],
                                    est[:, mi, :],
                                    start=(i == 0), stop=(i == len(kts) - 1),
                                )
                            for i, kt in enumerate(kts):
                                mi = kt - t + 1
                                nc.tensor.matmul(
                                    den[:, :P],
                                    ones_mat[:],
                                    est[:, mi, :],
                                    start=(i == 0), stop=(i == len(kts) - 1),
                                )
                            gated_norm(den, gb_sb, t * P, P, o_ps,
                                       OT[e][:, h, t * P : (t + 1) * P])

                else:  # linear
                    for h in range(nhc):
                        kv_ps = psB.tile([P, 512], f32, tag="ot", name="kv_ps")
                        ks_ps = psB.tile([P, 512], f32, tag="ot", name="ks_ps")
                        for st in range(HK):
                            nc.tensor.matmul(
                                kv_ps[:, :P],
                                KT[:, st, h * P : (h + 1) * P],
                                V[:, st, h * P : (h + 1) * P],
                                start=(st == 0), stop=(st == HK - 1),
                            )
                        for st in range(HK):
                            # ksum[d'] (column) = sum_s k'[s, d']
                            nc.tensor.matmul(
                                ks_ps[:, :1],
                                KT[:, st, h * P : (h + 1) * P],
                                ones_col[:, :],
                                start=(st == 0), stop=(st == HK - 1),
                            )
                        kv_sb = tpool.tile([P, P], bf16, tag="kv_sb", name="kv_sb")
                        nc.scalar.activation(kv_sb[:], kv_ps[:, :P], Copy)
                        # broadcast ksum column along free dim -> [P, P] lhsT
                        ks_bc = tpool.tile([P, P], bf16, tag="ks_bc", name="ks_bc")
                        nc.scalar.activation(
                            ks_bc[:], ks_ps[:, 0:1].to_broadcast([P, P]), Copy)
                        for (c0, cw) in chq:
                            num_ps = psA.tile([P, 512], f32, tag="mm", name="num_ps")
                            nc.tensor.matmul(
                                num_ps[:, :cw],
                                kv_sb[:],
                                QT[:, h, c0 : c0 + cw],
                                start=True, stop=True,
                            )
                            den = psC.tile([P, 512], f32, tag="den", name="den2")
                            nc.tensor.matmul(
                                den[:, :cw],
                                ks_bc[:],
                                QT[:, h, c0 : c0 + cw],
                                start=True, stop=True,
                            )
                            gated_norm(den, gb_sb, c0, cw, num_ps,
                                       OT[e][:, h, c0 : c0 + cw])

                # ---- per-expert out-projection (separate q-spaces)
                outproj(e, NQT if sparse else HK)

    nc.finalize()
    return nc


# ---------------------------------------------------------------- entry

def kernel(**inputs) -> np.ndarray:
    from concourse.bass_utils import run_bass_kernel_spmd

    inputs = {k: np.asarray(v, np.float32) if np.asarray(v).dtype.kind == "f"
              else np.asarray(v) for k, v in inputs.items()}
    masks = _band_masks()
    gatesT = [_host_gates(inputs["x"][b], inputs["Wg"]) for b in range(4)]
    sel_idx = [[np.nonzero(gatesT[b][e] > 0.0)[0] for e in range(E)]
               for b in range(4)]
    max_c = max(len(sel_idx[b][e]) for b in range(4) for e in SPARSE)
    s_pad = max(128, ((max_c + P - 1) // P) * P)
    bo_eff_all = np.stack([
        inputs[f"e{e}_bv"] @ inputs[f"e{e}_Wo"] + inputs[f"e{e}_bo"]
        for e in range(E)
    ])
    in_maps = [
        _prep_core(inputs, c // 2, c % 2, masks, gatesT[c // 2],
                   sel_idx[c // 2], s_pad)
        for c in range(N_CORES)
    ]
    nc = _build_nc(s_pad)
    trace = bool(int(os.environ.get("KERNEL_TRACE", "0")))
    if trace:
        import jax

        jax.devices()  # force axon platform registration
        try:
            from antenv.axon_hooks import (
                get_axon_ntff_profile_hook,
                set_axon_ntff_profile_hook,
            )

            if get_axon_ntff_profile_hook() is None:
                from trn_agent_boot.trn_boot import _ntff_profile_via_ctypes

                set_axon_ntff_profile_hook(
                    _ntff_profile_via_ctypes("/opt/axon/libaxon_pjrt.so"))
        except Exception as exc:  # tracing is best-effort
            print(f"NTFF hook setup failed: {exc}")
    res = run_bass_kernel_spmd(nc, in_maps, list(range(N_CORES)), trace=trace)
    if trace and res.exec_time_ns is not None:
        print(f"HW exec time: {res.exec_time_ns} ns")
    out = np.zeros((4, S, H), np.float32)
    for b in range(4):
        acc = (gatesT[b].T @ bo_eff_all).astype(np.float32)  # gated output bias
        for p in (0, 1):
            r = res.results[2 * b + p]
            for e in SPARSE:
                sel = sel_idx[b][e]
                acc[sel] += r[f"out{e}"][: len(sel)].astype(np.float32)
            acc += r["out3"].astype(np.float32)
        out[b] = acc
    return out
